# revision 11
# baseline (speedup 1.0000x reference)
"""Trainium2 Bass kernel for nn_Air_Model (Elman RNN cell over L=512 steps).

reference:
    ux = einsum("bln,ns->bls", x, U_w) + U_b          # [B, L, S]
    scan over l: a = relu(ux_l + a @ W_w + W_b)       # a: [B, S]
    out = a_last @ V_w + V_b                          # [B, M]

Shapes: B=4096, L=512, N=12, S=128, M=12 (fp32 in/out).

Strategy (data-parallel over batch, 8 cores, B_local=512 per core):
  - Host-side prep: x is pre-transposed/cast to bf16 [L, 13, B_local] with a
    baked ones-row (row 12) so the U-matmul picks up the combined U_b+W_b
    bias; a0 pre-transposed to [S, B_local] bf16; U replicated at the four
    32-row groups of an augmented [128, S] weight tile. This removes every
    on-device transpose/cast from the baseline (~150us of PE + ~90us DVE).
  - Scan state lives transposed in SBUF as bf16 [S=128 part, B free], split
    into 4 independent chains (widths CHAINS) so the serial
    mm -> relu -> mm latency is amortized across narrower tiles; relus run
    on DVE for the wide chains and ScalarE for the narrow ones.
  - Per step l: the U-matmuls for step l+1 (K=13, row group 32*(l%4)) are
    issued BEFORE the W-matmuls of step l into per-chain full-bank PSUM
    tiles (bufs=2), keeping the bias/input projection off the critical
    path. A BIR post-pass drops back-to-back identical LDWEIGHTS so the 4
    W-matmuls (and the 4 U-matmuls) per step share one weight load.
  - Final projection uses the state directly as lhsT: out = A^T.T @ V_w,
    with V_b folded in via a K=1 ones-row matmul.

The BIR post-passes below work around walrus/Tile mismatches in this
container (single sync-wait-per-instruction walrus; dependency waits landing
on weight loads).
"""

import numpy as np

import concourse.bass as bass
import concourse.mybir as mybir
import concourse.tile as tile
from bass_rust import InstructionNameOrderedSet
from concourse.bass_utils import run_bass_kernel_spmd
from concourse.vector_clock import ScopedClock
from bass_rust import SemaphoreHandle

# ---------------------------------------------------------------------------
# Patch: this walrus build supports only ONE sync-wait per instruction, but
# Tile's kernel-tail drain accumulates one wait per outstanding semaphore.
# Split them into one drain instruction per wait.
# ---------------------------------------------------------------------------


def _drain_and_barrier_split(self, tick_clock, wait_clock):
    nc = self.nc
    probe = mybir.InstDrain(name=nc.get_next_instruction_name(), ins=[], outs=[])
    probe.engine = mybir.EngineType.SP
    wait_clock.add_sem_waits(probe, ScopedClock({None: tick_clock.global_clock}))
    waits = list(probe.sync_info.on_wait) if probe.sync_info else []
    for w in waits:
        d = nc.sync.drain()
        sem = SemaphoreHandle(num=w.id, name=w.ant_name)
        d.wait_op(sem, w.wait_value, w.wait_mode.removesuffix("-imm"))
    if not waits:
        nc.sync.drain()

    nc.all_engine_barrier()
    assert self.sems is not None
    popped = nc._tile_sem_poison_stack.pop()
    assert popped is self._sem_poison
    nc.clear_and_free_semaphores(list(self.sems.allocated().values()))


tile.TileContext._drain_and_barrier = _drain_and_barrier_split


def _split_multi_waits(nc):
    """Walrus here allows only one sync-wait per instruction, but Tile's
    semaphore assignment can attach several. Hoist extra waits onto fresh
    NOPs placed immediately before the instruction on the same engine."""
    import bass_rust

    SyncInfo = bass_rust.SyncInfo
    n_split = 0
    for fn in nc.m.functions:
        for blk in fn.blocks:
            insts = blk.instructions
            if not any(
                i.sync_info is not None and len(i.sync_info.on_wait) > 1
                for i in insts
            ):
                continue
            new = []
            for inst in insts:
                si = inst.sync_info
                if si is not None and len(si.on_wait) > 1:
                    waits = list(si.on_wait)
                    for w in waits[:-1]:
                        nop = mybir.InstNoOp(
                            name=nc.get_next_instruction_name(), ins=[], outs=[]
                        )
                        nop.engine = inst.engine
                        nop.sync_info = SyncInfo(on_wait=[w], on_update=[])
                        new.append(nop)
                        n_split += 1
                    inst.sync_info = SyncInfo(
                        on_wait=[waits[-1]], on_update=list(si.on_update)
                    )
                new.append(inst)
            blk.instructions = new
    return n_split


def _unblock_param_ldweights(nc):
    """Walrus/Tile put the scan dependency wait on the LDWEIGHTS of each
    matmul, serializing the (constant) weight load behind the wait. For
    weight tiles that are write-once (W/U params), strip the waits off the
    LDWEIGHTS and re-attach them to a NOP between it and the matmul: the
    weight load can then run ahead while the wait only gates the matmul."""
    import bass_rust

    SyncInfo = bass_rust.SyncInfo
    moved = 0
    for fn in nc.m.functions:
        for blk in fn.blocks:
            insts = blk.instructions
            new = []
            for inst in insts:
                new.append(inst)
                if (
                    type(inst).__name__ == "InstLdweights"
                    and inst.sync_info is not None
                    and inst.sync_info.on_wait
                    and inst.ins
                    and getattr(inst.ins[0], "memref", "").startswith(("w_sb", "u_sb"))
                ):
                    si = inst.sync_info
                    nop = mybir.InstNoOp(
                        name=nc.get_next_instruction_name(), ins=[], outs=[]
                    )
                    nop.engine = inst.engine
                    nop.sync_info = SyncInfo(
                        on_wait=list(si.on_wait), on_update=[]
                    )
                    inst.sync_info = SyncInfo(
                        on_wait=[], on_update=list(si.on_update)
                    )
                    new.append(nop)
                    moved += 1
            blk.instructions = new
    return moved


def _dedup_ldweights(nc):
    """Per scan step the four W matmuls (and the four U matmuls) load
    identical weights back-to-back. Drop an InstLdweights when the previous
    weight load in the same block is bit-identical and nothing else reloaded
    the array."""
    dropped = 0
    for fn in nc.m.functions:
        for blk in fn.blocks:
            new = []
            last_sig = None
            for inst in blk.instructions:
                tn = type(inst).__name__
                if tn == "InstLdweights":
                    a = inst.ins[0]
                    sig = (getattr(a, "memref", None), a.offset, str(a.ap))
                    has_sync = inst.sync_info is not None and (
                        inst.sync_info.on_wait or inst.sync_info.on_update
                    )
                    if sig == last_sig and not has_sync:
                        dropped += 1
                        continue
                    last_sig = sig
                new.append(inst)
            blk.instructions = new
    return dropped


# ---------------------------------------------------------------------------

B, L, N, S, M = 4096, 512, 12, 128, 12
NCORES = 8
BL = B // NCORES        # 512 local batch
NP = N + 1              # 13: n rows + ones row for the bias
NG = L // 4             # x tile groups (4 steps per [NP, 4*BL] tile)
PF = 8                  # x groups prefetched ahead

# (col offset, width, relu engine) per scan chain; widths sum to BL
CHAINS = [
    (0, 176, "dve"),
    (176, 176, "dve"),
    (352, 96, "act"),
    (448, 64, "act"),
]
WORDER = [0, 1, 2, 3]   # large chains first; block ends small (shorter turn)

F32 = mybir.dt.float32
BF16 = mybir.dt.bfloat16
AF = mybir.ActivationFunctionType
ALU = mybir.AluOpType


def _build():
    nc = bass.Bass(trn_type="TRN2")

    x_d = nc.dram_tensor("xt", [NG, NP, 4 * BL], BF16, kind="ExternalInput")
    a0_d = nc.dram_tensor("a0t", [S, BL], BF16, kind="ExternalInput")
    u_d = nc.dram_tensor("uaug", [NP, S], BF16, kind="ExternalInput")
    w_d = nc.dram_tensor("wmat", [S, S], BF16, kind="ExternalInput")
    v_d = nc.dram_tensor("vw", [S, M], BF16, kind="ExternalInput")
    vb_d = nc.dram_tensor("vb", [1, M], BF16, kind="ExternalInput")
    out_d = nc.dram_tensor("out", [BL, M], F32, kind="ExternalOutput")

    with tile.TileContext(nc) as tc:
        with (
            tc.tile_pool(name="xpool", bufs=PF) as xpool,
            tc.tile_pool(name="singles", bufs=1) as singles,
            tc.tile_pool(name="ps", bufs=2, space="PSUM") as ps,
        ):
            # ---- x streaming: one [NP, 4*BL] tile covers 4 steps (step
            # l=4t+g owns columns g*BL..(g+1)*BL) -----------------------------
            xtiles = {}

            def fetch_group(t):
                xg = xpool.tile([NP, 4 * BL], BF16, tag="xg", name="xg")
                nc.sync.dma_start(out=xg, in_=x_d[t, :, :])
                xtiles[t] = xg

            for t in range(PF):
                fetch_group(t)

            # ---- parameters (already laid out host-side) -------------------
            w_sb = singles.tile([S, S], BF16, tag="w", name="w_sb")
            nc.sync.dma_start(out=w_sb, in_=w_d[:, :])
            u_sb = singles.tile([NP, S], BF16, tag="u", name="u_sb")
            nc.sync.dma_start(out=u_sb, in_=u_d[:, :])
            v_sb = singles.tile([S, M], BF16, tag="v", name="v_sb")
            nc.sync.dma_start(out=v_sb, in_=v_d[:, :])
            vb_sb = singles.tile([1, M], BF16, tag="vb", name="vb_sb")
            nc.sync.dma_start(out=vb_sb, in_=vb_d[:, :])
            ones_row = singles.tile([1, 128], BF16, tag="ones", name="ones_row")
            nc.vector.memset(ones_row, 1.0)

            # ---- scan state A^T: tile per (parity, chain) ------------------
            a_t = [
                [
                    singles.tile([S, w], BF16, tag=f"a{i}_{c}", name=f"a{i}_{c}")
                    for c, (off, w, eng) in enumerate(CHAINS)
                ]
                for i in range(2)
            ]
            for c, (off, w, eng) in enumerate(CHAINS):
                nc.sync.dma_start(out=a_t[0][c], in_=a0_d[:, off : off + w])

            def new_psums():
                return [
                    ps.tile([128, 512], F32, tag=f"pc{c}", name=f"pc{c}")
                    for c in range(len(CHAINS))
                ]

            def u_mms(l, into, after):
                """U-projection matmuls for step l (PSUM prefill). `after` is
                an instruction name the block is nosync-ordered behind so the
                PE stream stays [W-block | U-block | W-block ...] and the
                identical LDWEIGHTS within each block dedup."""
                t, g = l // 4, l % 4
                xg = xtiles[t]
                last = None
                for c, (off, w, eng) in enumerate(CHAINS):
                    mi = nc.tensor.matmul(
                        into[c][:, 0:w],
                        u_sb,
                        xg[:, g * BL + off : g * BL + off + w],
                        start=True,
                        stop=False,
                    )
                    if after is not None:
                        mi.ins.add_nosync_dependencies_from(InstructionNameOrderedSet([after]))
                    last = mi.ins.name
                return last

            # ---- main loop -------------------------------------------------
            # PE block order:  W(0) | U(1) U(2) | W(1) W(2) | U(3) U(4) | ...
            # U-blocks run on even steps only and prefill TWO steps: U(l+1)
            # reuses the bank relu(l-1) freed, U(l+2) the bank relu(l) frees
            # mid-block. Odd steps then run W back-to-back on still-loaded
            # weights, halving both the LDW count and the array-drain turns.
            psums = {}

            def new_psums(l):
                psums[l] = [
                    ps.tile([128, 512], F32, tag=f"pc{c}", name=f"pc{c}")
                    for c in range(len(CHAINS))
                ]

            new_psums(0)
            u_last = u_mms(0, psums[0], None)
            for l in range(L):
                if l % 4 == 0:
                    t = l // 4
                    xtiles.pop(t - 1, None)
                    if t + PF < NG:
                        fetch_group(t + PF)
                a_prev = a_t[l % 2]
                a_new = a_t[(l + 1) % 2]
                ps_cur = psums.pop(l)
                w_last = None
                for c in WORDER:
                    off, w, eng = CHAINS[c]
                    wi = nc.tensor.matmul(
                        ps_cur[c][:, 0:w], w_sb, a_prev[c], start=False, stop=True
                    )
                    if u_last is not None:
                        wi.ins.add_nosync_dependencies_from(
                            InstructionNameOrderedSet([u_last])
                        )
                    w_last = wi.ins.name
                for c in WORDER:
                    off, w, eng = CHAINS[c]
                    if eng == "act":
                        nc.scalar.activation(
                            a_new[c], ps_cur[c][:, 0:w], AF.Relu, bias=0.0, scale=1.0
                        )
                    else:
                        nc.vector.tensor_scalar(
                            out=a_new[c],
                            in0=ps_cur[c][:, 0:w],
                            scalar1=0.0,
                            scalar2=None,
                            op0=ALU.max,
                        )
                if l + 1 < L:
                    new_psums(l + 1)
                    u_last = u_mms(l + 1, psums[l + 1], w_last)

            # ---- output: out[b, m] = A^T.T @ V_w + V_b ---------------------
            a_last = a_t[L % 2]
            afull = singles.tile([S, BL], BF16, tag="afull", name="afull")
            for c, (off, w, eng) in enumerate(CHAINS):
                nc.vector.tensor_copy(afull[:, off : off + w], a_last[c])
            for cb in range(BL // 128):
                po = ps.tile([128, 512], F32, tag=f"pc{cb}", name=f"pc{cb}")
                nc.tensor.matmul(
                    po[:, 0:M], ones_row, vb_sb, start=True, stop=False
                )
                nc.tensor.matmul(
                    po[:, 0:M],
                    afull[:, cb * 128 : (cb + 1) * 128],
                    v_sb,
                    start=False,
                    stop=True,
                )
                o_sb = singles.tile([128, M], F32, tag=f"osb{cb}", name=f"osb{cb}")
                nc.scalar.copy(out=o_sb, in_=po[:, 0:M])
                nc.sync.dma_start(
                    out=out_d[cb * 128 : (cb + 1) * 128, :], in_=o_sb
                )

    _unblock_param_ldweights(nc)
    _dedup_ldweights(nc)
    _split_multi_waits(nc)
    return nc


_CACHED_NC = None


def _get_nc():
    global _CACHED_NC
    if _CACHED_NC is None:
        _CACHED_NC = _build()
    return _CACHED_NC


def _prep_in_maps(inputs):
    """Host-side reshape/cast: transpose x and a0 into the device layouts,
    fold the biases into an augmented U weight tile, cast params to bf16."""
    import ml_dtypes

    bf16 = ml_dtypes.bfloat16

    x = np.asarray(inputs["x"], dtype=np.float32)
    a0 = np.asarray(inputs["a0"], dtype=np.float32)
    U_w = np.asarray(inputs["U_w"], dtype=np.float32)
    U_b = np.asarray(inputs["U_b"], dtype=np.float32)
    W_w = np.asarray(inputs["W_w"], dtype=np.float32)
    W_b = np.asarray(inputs["W_b"], dtype=np.float32)
    V_w = np.asarray(inputs["V_w"], dtype=np.float32)
    V_b = np.asarray(inputs["V_b"], dtype=np.float32)

    # [NCORES, NG, NP, 4, BL] with ones in row N; step l=4t+g owns
    # columns g*BL..(g+1)*BL of group t's [NP, 4*BL] tile
    xt = np.empty((NCORES, NG, NP, 4, BL), dtype=bf16)
    xt[:, :, :N, :, :] = (
        x.reshape(NCORES, BL, NG, 4, N).transpose(0, 2, 4, 3, 1).astype(bf16)
    )
    xt[:, :, N, :, :] = np.asarray(1.0, dtype=bf16)
    xt = xt.reshape(NCORES, NG, NP, 4 * BL)
    a0t = a0.reshape(NCORES, BL, S).transpose(0, 2, 1).astype(bf16)

    uaug = np.empty((NP, S), dtype=np.float32)
    uaug[:N, :] = U_w
    uaug[N, :] = U_b + W_b
    uaug = uaug.astype(bf16)
    wmat = W_w.astype(bf16)
    vw = V_w.astype(bf16)
    vb = V_b[None, :].astype(bf16)

    in_maps = []
    for i in range(NCORES):
        in_maps.append(
            {
                "xt": np.ascontiguousarray(xt[i]),
                "a0t": np.ascontiguousarray(a0t[i]),
                "uaug": uaug,
                "wmat": wmat,
                "vw": vw,
                "vb": vb,
            }
        )
    return in_maps


def kernel(**inputs):
    nc = _get_nc()
    in_maps = _prep_in_maps(inputs)
    res = run_bass_kernel_spmd(nc, in_maps, core_ids=list(range(NCORES)))
    out = np.concatenate([res.results[i]["out"] for i in range(NCORES)], axis=0)
    return out.astype(np.float32)


# revision 16
# speedup vs baseline: 1.3811x; 1.3811x over previous
"""Trainium2 Bass kernel for nn_Air_Model (Elman RNN cell over L=512 steps).

reference:
    ux = einsum("bln,ns->bls", x, U_w) + U_b          # [B, L, S]
    scan over l: a = relu(ux_l + a @ W_w + W_b)       # a: [B, S]
    out = a_last @ V_w + V_b                          # [B, M]

Shapes: B=4096, L=512, N=12, S=128, M=12 (fp32 in/out).

Strategy (data-parallel over batch, 8 cores, B_local=512 per core):
  - Host-side prep: x is pre-transposed/cast to bf16 tiles [13, 4*B_local]
    (one tile per 4 steps, ones baked into row 12 so the K=13 U-matmul
    picks up the combined U_b+W_b bias); a0 pre-transposed to [S, B_local]
    bf16; U_w/U_b+W_b packed into one [13, S] weight tile. This removes
    every on-device transpose/cast the original kernel needed (~150us of
    PE + ~90us of DVE per core).
  - Scan state lives transposed in SBUF as bf16 [S=128 part, B free], split
    into 4 independent chains (CHAINS widths; one PSUM bank per chain per
    parity = all 8 banks) so the serial mm -> relu -> mm latency is
    amortized across narrower tiles; relus run on DVE for the wide chains
    and ScalarE for the narrow ones.
  - Per step l the PE stream is [W-block(l) | U-block(l+1)]: the U-matmuls
    prefill the next step's banks while the relus of step l drain, kept in
    that order with nosync scheduling deps so the 4 W-matmuls (and the 4
    U-matmuls) of each block share one LDWEIGHTS after the dedup post-pass
    (2 weight loads + 2 array-drain turns per step; batching U two steps
    ahead was measured slower - the relu latency surfaces as a PE hole
    between consecutive W-blocks).
  - Final projection uses the state directly as lhsT: out = A^T.T @ V_w,
    with V_b folded in via a K=1 ones-row matmul.

Measured on 8 axon-tunneled trn2 NeuronCores: ~0.58 ms HW exec time
(baseline kernel: 0.77-0.92 ms), max relative error ~3.9e-3 vs the fp32
jax reference. Steady state is PE-issue-bound at ~1.07 us/step: 426 ns of
rhs streaming + ~200 ns LDWEIGHTS + ~340 ns of array-drain turns between
the U/W weight swaps + per-mm fixed costs; the W<->U weight swap cannot be
avoided because W fills all 128 PE rows (K=128+13 > 128).

The BIR post-passes below work around walrus/Tile mismatches in this
container (single sync-wait-per-instruction walrus; dependency waits landing
on weight loads -- the first load of each weight tile keeps its wait so the
array never races the parameter DMA).
"""

import numpy as np

import concourse.bass as bass
import concourse.mybir as mybir
import concourse.tile as tile
from bass_rust import InstructionNameOrderedSet
from concourse.bass_utils import run_bass_kernel_spmd
from concourse.vector_clock import ScopedClock
from bass_rust import SemaphoreHandle

# ---------------------------------------------------------------------------
# Patch: this walrus build supports only ONE sync-wait per instruction, but
# Tile's kernel-tail drain accumulates one wait per outstanding semaphore.
# Split them into one drain instruction per wait.
# ---------------------------------------------------------------------------


def _drain_and_barrier_split(self, tick_clock, wait_clock):
    nc = self.nc
    probe = mybir.InstDrain(name=nc.get_next_instruction_name(), ins=[], outs=[])
    probe.engine = mybir.EngineType.SP
    wait_clock.add_sem_waits(probe, ScopedClock({None: tick_clock.global_clock}))
    waits = list(probe.sync_info.on_wait) if probe.sync_info else []
    for w in waits:
        d = nc.sync.drain()
        sem = SemaphoreHandle(num=w.id, name=w.ant_name)
        d.wait_op(sem, w.wait_value, w.wait_mode.removesuffix("-imm"))
    if not waits:
        nc.sync.drain()

    nc.all_engine_barrier()
    assert self.sems is not None
    popped = nc._tile_sem_poison_stack.pop()
    assert popped is self._sem_poison
    nc.clear_and_free_semaphores(list(self.sems.allocated().values()))


tile.TileContext._drain_and_barrier = _drain_and_barrier_split


def _split_multi_waits(nc):
    """Walrus here allows only one sync-wait per instruction, but Tile's
    semaphore assignment can attach several. Hoist extra waits onto fresh
    NOPs placed immediately before the instruction on the same engine."""
    import bass_rust

    SyncInfo = bass_rust.SyncInfo
    n_split = 0
    for fn in nc.m.functions:
        for blk in fn.blocks:
            insts = blk.instructions
            if not any(
                i.sync_info is not None and len(i.sync_info.on_wait) > 1
                for i in insts
            ):
                continue
            new = []
            for inst in insts:
                si = inst.sync_info
                if si is not None and len(si.on_wait) > 1:
                    waits = list(si.on_wait)
                    for w in waits[:-1]:
                        nop = mybir.InstNoOp(
                            name=nc.get_next_instruction_name(), ins=[], outs=[]
                        )
                        nop.engine = inst.engine
                        nop.sync_info = SyncInfo(on_wait=[w], on_update=[])
                        new.append(nop)
                        n_split += 1
                    inst.sync_info = SyncInfo(
                        on_wait=[waits[-1]], on_update=list(si.on_update)
                    )
                new.append(inst)
            blk.instructions = new
    return n_split


def _unblock_param_ldweights(nc):
    """Walrus/Tile put the scan dependency wait on the LDWEIGHTS of each
    matmul, serializing the (constant) weight load behind the wait. For
    weight tiles that are write-once (W/U params), strip the waits off the
    LDWEIGHTS and re-attach them to a NOP between it and the matmul: the
    weight load can then run ahead while the wait only gates the matmul."""
    import bass_rust

    SyncInfo = bass_rust.SyncInfo
    moved = 0
    seen_memrefs = set()
    for fn in nc.m.functions:
        for blk in fn.blocks:
            insts = blk.instructions
            new = []
            for inst in insts:
                new.append(inst)
                if (
                    type(inst).__name__ == "InstLdweights"
                    and inst.ins
                    and getattr(inst.ins[0], "memref", "").startswith(("w_sb", "u_sb"))
                    and inst.ins[0].memref not in seen_memrefs
                ):
                    # keep the FIRST load of each weight tile fully gated: its
                    # wait covers the parameter DMA; stripping it lets the
                    # array load race the DMA (intermittent garbage weights)
                    seen_memrefs.add(inst.ins[0].memref)
                    continue
                if (
                    type(inst).__name__ == "InstLdweights"
                    and inst.sync_info is not None
                    and inst.sync_info.on_wait
                    and inst.ins
                    and getattr(inst.ins[0], "memref", "").startswith(("w_sb", "u_sb"))
                ):
                    si = inst.sync_info
                    nop = mybir.InstNoOp(
                        name=nc.get_next_instruction_name(), ins=[], outs=[]
                    )
                    nop.engine = inst.engine
                    nop.sync_info = SyncInfo(
                        on_wait=list(si.on_wait), on_update=[]
                    )
                    inst.sync_info = SyncInfo(
                        on_wait=[], on_update=list(si.on_update)
                    )
                    new.append(nop)
                    moved += 1
            blk.instructions = new
    return moved


def _dedup_ldweights(nc):
    """Per scan step the four W matmuls (and the four U matmuls) load
    identical weights back-to-back. Drop an InstLdweights when the previous
    weight load in the same block is bit-identical and nothing else reloaded
    the array."""
    dropped = 0
    for fn in nc.m.functions:
        for blk in fn.blocks:
            new = []
            last_sig = None
            for inst in blk.instructions:
                tn = type(inst).__name__
                if tn == "InstLdweights":
                    a = inst.ins[0]
                    sig = (getattr(a, "memref", None), a.offset, str(a.ap))
                    has_sync = inst.sync_info is not None and (
                        inst.sync_info.on_wait or inst.sync_info.on_update
                    )
                    if sig == last_sig and not has_sync:
                        dropped += 1
                        continue
                    last_sig = sig
                new.append(inst)
            blk.instructions = new
    return dropped


# ---------------------------------------------------------------------------

B, L, N, S, M = 4096, 512, 12, 128, 12
NCORES = 8
BL = B // NCORES        # 512 local batch
NP = N + 1              # 13: n rows + ones row for the bias
NG = L // 4             # x tile groups (4 steps per [NP, 4*BL] tile)
PF = 8                  # x groups prefetched ahead

# (col offset, width, relu engine) per scan chain; widths sum to BL
CHAINS = [
    (0, 176, "dve"),
    (176, 176, "dve"),
    (352, 96, "act"),
    (448, 64, "act"),
]
WORDER = [2, 3, 0, 1]   # ACT chains' W-matmuls first (their relu is slower)

F32 = mybir.dt.float32
BF16 = mybir.dt.bfloat16
AF = mybir.ActivationFunctionType
ALU = mybir.AluOpType


def _build():
    nc = bass.Bass(trn_type="TRN2")

    x_d = nc.dram_tensor("xt", [NG, NP, 4 * BL], BF16, kind="ExternalInput")
    a0_d = nc.dram_tensor("a0t", [S, BL], BF16, kind="ExternalInput")
    u_d = nc.dram_tensor("uaug", [NP, S], BF16, kind="ExternalInput")
    w_d = nc.dram_tensor("wmat", [S, S], BF16, kind="ExternalInput")
    v_d = nc.dram_tensor("vw", [S, M], BF16, kind="ExternalInput")
    vb_d = nc.dram_tensor("vb", [1, M], BF16, kind="ExternalInput")
    out_d = nc.dram_tensor("out", [BL, M], F32, kind="ExternalOutput")

    with tile.TileContext(nc) as tc:
        with (
            tc.tile_pool(name="xpool", bufs=PF) as xpool,
            tc.tile_pool(name="singles", bufs=1) as singles,
            tc.tile_pool(name="ps", bufs=2, space="PSUM") as ps,
        ):
            # ---- x streaming: one [NP, 4*BL] tile covers 4 steps (step
            # l=4t+g owns columns g*BL..(g+1)*BL) -----------------------------
            xtiles = {}

            def fetch_group(t):
                xg = xpool.tile([NP, 4 * BL], BF16, tag="xg", name="xg")
                nc.sync.dma_start(out=xg, in_=x_d[t, :, :])
                xtiles[t] = xg

            for t in range(PF):
                fetch_group(t)

            # ---- parameters (already laid out host-side) -------------------
            w_sb = singles.tile([S, S], BF16, tag="w", name="w_sb")
            nc.sync.dma_start(out=w_sb, in_=w_d[:, :])
            u_sb = singles.tile([NP, S], BF16, tag="u", name="u_sb")
            nc.sync.dma_start(out=u_sb, in_=u_d[:, :])
            v_sb = singles.tile([S, M], BF16, tag="v", name="v_sb")
            nc.sync.dma_start(out=v_sb, in_=v_d[:, :])
            vb_sb = singles.tile([1, M], BF16, tag="vb", name="vb_sb")
            nc.sync.dma_start(out=vb_sb, in_=vb_d[:, :])
            ones_row = singles.tile([1, 128], BF16, tag="ones", name="ones_row")
            nc.vector.memset(ones_row, 1.0)

            # ---- scan state A^T: tile per (parity, chain) ------------------
            a_t = [
                [
                    singles.tile([S, w], BF16, tag=f"a{i}_{c}", name=f"a{i}_{c}")
                    for c, (off, w, eng) in enumerate(CHAINS)
                ]
                for i in range(2)
            ]
            for c, (off, w, eng) in enumerate(CHAINS):
                nc.sync.dma_start(out=a_t[0][c], in_=a0_d[:, off : off + w])

            def new_psums():
                return [
                    ps.tile([128, 512], F32, tag=f"pc{c}", name=f"pc{c}")
                    for c in range(len(CHAINS))
                ]

            def u_mms(l, into, after):
                """U-projection matmuls for step l (PSUM prefill). `after` is
                an instruction name the block is nosync-ordered behind so the
                PE stream stays [W-block | U-block | W-block ...] and the
                identical LDWEIGHTS within each block dedup."""
                t, g = l // 4, l % 4
                xg = xtiles[t]
                last = None
                for c, (off, w, eng) in enumerate(CHAINS):
                    mi = nc.tensor.matmul(
                        into[c][:, 0:w],
                        u_sb,
                        xg[:, g * BL + off : g * BL + off + w],
                        start=True,
                        stop=False,
                    )
                    if after is not None:
                        mi.ins.add_nosync_dependencies_from(InstructionNameOrderedSet([after]))
                    last = mi.ins.name
                return last

            # ---- main loop -------------------------------------------------
            # PE block order:  W(0) | U(1) U(2) | W(1) W(2) | U(3) U(4) | ...
            # U-blocks run on even steps only and prefill TWO steps: U(l+1)
            # reuses the bank relu(l-1) freed, U(l+2) the bank relu(l) frees
            # mid-block. Odd steps then run W back-to-back on still-loaded
            # weights, halving both the LDW count and the array-drain turns.
            psums = {}

            def new_psums(l):
                psums[l] = [
                    ps.tile([128, 512], F32, tag=f"pc{c}", name=f"pc{c}")
                    for c in range(len(CHAINS))
                ]

            new_psums(0)
            u_last = u_mms(0, psums[0], None)
            for l in range(L):
                if l % 4 == 0:
                    t = l // 4
                    xtiles.pop(t - 1, None)
                    if t + PF < NG:
                        fetch_group(t + PF)
                a_prev = a_t[l % 2]
                a_new = a_t[(l + 1) % 2]
                ps_cur = psums.pop(l)
                w_last = None
                for c in WORDER:
                    off, w, eng = CHAINS[c]
                    wi = nc.tensor.matmul(
                        ps_cur[c][:, 0:w], w_sb, a_prev[c], start=False, stop=True
                    )
                    if u_last is not None:
                        wi.ins.add_nosync_dependencies_from(
                            InstructionNameOrderedSet([u_last])
                        )
                    w_last = wi.ins.name
                for c in WORDER:
                    off, w, eng = CHAINS[c]
                    if eng == "act":
                        nc.scalar.activation(
                            a_new[c], ps_cur[c][:, 0:w], AF.Relu, bias=0.0, scale=1.0
                        )
                    else:
                        nc.vector.tensor_scalar(
                            out=a_new[c],
                            in0=ps_cur[c][:, 0:w],
                            scalar1=0.0,
                            scalar2=None,
                            op0=ALU.max,
                        )
                if l + 1 < L:
                    new_psums(l + 1)
                    u_last = u_mms(l + 1, psums[l + 1], w_last)

            # ---- output: out[b, m] = A^T.T @ V_w + V_b ---------------------
            a_last = a_t[L % 2]
            afull = singles.tile([S, BL], BF16, tag="afull", name="afull")
            for c, (off, w, eng) in enumerate(CHAINS):
                nc.vector.tensor_copy(afull[:, off : off + w], a_last[c])
            for cb in range(BL // 128):
                po = ps.tile([128, 512], F32, tag=f"pc{cb}", name=f"pc{cb}")
                nc.tensor.matmul(
                    po[:, 0:M], ones_row, vb_sb, start=True, stop=False
                )
                nc.tensor.matmul(
                    po[:, 0:M],
                    afull[:, cb * 128 : (cb + 1) * 128],
                    v_sb,
                    start=False,
                    stop=True,
                )
                o_sb = singles.tile([128, M], F32, tag=f"osb{cb}", name=f"osb{cb}")
                nc.scalar.copy(out=o_sb, in_=po[:, 0:M])
                nc.sync.dma_start(
                    out=out_d[cb * 128 : (cb + 1) * 128, :], in_=o_sb
                )

    _unblock_param_ldweights(nc)
    _dedup_ldweights(nc)
    _split_multi_waits(nc)
    return nc


_CACHED_NC = None


def _get_nc():
    global _CACHED_NC
    if _CACHED_NC is None:
        _CACHED_NC = _build()
    return _CACHED_NC


def _prep_in_maps(inputs):
    """Host-side reshape/cast: transpose x and a0 into the device layouts,
    fold the biases into an augmented U weight tile, cast params to bf16."""
    import ml_dtypes

    bf16 = ml_dtypes.bfloat16

    x = np.asarray(inputs["x"], dtype=np.float32)
    a0 = np.asarray(inputs["a0"], dtype=np.float32)
    U_w = np.asarray(inputs["U_w"], dtype=np.float32)
    U_b = np.asarray(inputs["U_b"], dtype=np.float32)
    W_w = np.asarray(inputs["W_w"], dtype=np.float32)
    W_b = np.asarray(inputs["W_b"], dtype=np.float32)
    V_w = np.asarray(inputs["V_w"], dtype=np.float32)
    V_b = np.asarray(inputs["V_b"], dtype=np.float32)

    # [NCORES, NG, NP, 4, BL] with ones in row N; step l=4t+g owns
    # columns g*BL..(g+1)*BL of group t's [NP, 4*BL] tile
    xt = np.empty((NCORES, NG, NP, 4, BL), dtype=bf16)
    xt[:, :, :N, :, :] = (
        x.reshape(NCORES, BL, NG, 4, N).transpose(0, 2, 4, 3, 1).astype(bf16)
    )
    xt[:, :, N, :, :] = np.asarray(1.0, dtype=bf16)
    xt = xt.reshape(NCORES, NG, NP, 4 * BL)
    a0t = a0.reshape(NCORES, BL, S).transpose(0, 2, 1).astype(bf16)

    uaug = np.empty((NP, S), dtype=np.float32)
    uaug[:N, :] = U_w
    uaug[N, :] = U_b + W_b
    uaug = uaug.astype(bf16)
    wmat = W_w.astype(bf16)
    vw = V_w.astype(bf16)
    vb = V_b[None, :].astype(bf16)

    in_maps = []
    for i in range(NCORES):
        in_maps.append(
            {
                "xt": np.ascontiguousarray(xt[i]),
                "a0t": np.ascontiguousarray(a0t[i]),
                "uaug": uaug,
                "wmat": wmat,
                "vw": vw,
                "vb": vb,
            }
        )
    return in_maps


def kernel(**inputs):
    nc = _get_nc()
    in_maps = _prep_in_maps(inputs)
    res = run_bass_kernel_spmd(nc, in_maps, core_ids=list(range(NCORES)))
    out = np.concatenate([res.results[i]["out"] for i in range(NCORES)], axis=0)
    return out.astype(np.float32)
